# revision 10
# baseline (speedup 1.0000x reference)
"""Trainium2 Bass kernel for nn_APPM_996432413602 (nms_detection).

Pure data-parallel over batch: 64 batches -> 8 cores x 8 batches.

Per-core pipeline:
  1. DMA x shard [8, 2048, 14, 14] in per-batch tiles [128, 16, 196]
     (partition p holds channels 16p..16p+15).
  2. Channel sum: DVE pairwise-folds chunks r=0..7 in-place; PE accumulates
     chunks r=8..15 plus the folded chunk with ones-matmuls into psum [1,196].
  3. Window scores: PE-transpose s [8,196] -> [196,8], then fp32 matmuls with
     the 0/(1/area) pooling matrix -> scores [8, 917].
  4. Greedy NMS, all (group, batch) pairs at once in a [96, 384] layout
     (row 32g+b = group g / batch b, cols = group-local window, pad -1e30):
     3 rounds of max8 -> max_index -> indirect-DMA gather of the picked
     window's precomputed suppression row (-1e30 at IoU>0.25 or self) -> add.
"""

import numpy as np

# ---------------------------------------------------------------------------
# Problem constants (hardcoded from the reference)
# ---------------------------------------------------------------------------
STRIDE = 32
SIZE = 14
RATIOS = [[4, 4], [3, 5], [5, 3], [6, 6], [5, 7], [7, 5], [8, 8], [6, 10],
          [10, 6], [7, 9], [9, 7], [7, 10], [10, 7]]
N_LIST = [2, 3, 2]
IOU_THRESH = 0.25
WINDOW_NUMS = [(SIZE - rh + 1) * (SIZE - rw + 1) for (rh, rw) in RATIOS]
GROUP_BOUNDS = [0, 361, 602, 917]
W_G = [361, 241, 315]
WPAD = 384          # padded per-group window axis
NROWS = 96          # 3 groups x 32 rows (8 used per group)
NB = 8              # batches per core
NCORES = 8
NEG = -1.0e30

B_TOT, C, H, W = 64, 2048, 14, 14
HW = H * W          # 196


def _make_coords():
    coords = []
    for (rh, rw) in RATIOS:
        hw = SIZE - rh + 1
        ww = SIZE - rw + 1
        idx = np.arange(hw * ww)
        xi = idx // ww
        yi = idx % ww
        x1 = xi * STRIDE - 1
        y1 = yi * STRIDE - 1
        x2 = x1 + rh * STRIDE
        y2 = y1 + rw * STRIDE
        x1 = np.maximum(x1, 0)
        y1 = np.maximum(y1, 0)
        coords.append(np.stack([x1, y1, x2, y2], axis=1))
    return np.concatenate(coords, axis=0).astype(np.float32)


def _build_consts():
    coords = _make_coords()                                   # [917, 4] f32
    areas = ((coords[:, 2] - coords[:, 0] + np.float32(1.0))
             * (coords[:, 3] - coords[:, 1] + np.float32(1.0)))  # f32

    # Pooling matrix: Wm[p, w] = 1/(rh*rw) if cell p inside window w.
    wm = np.zeros((HW, 917), dtype=np.float32)
    wbase = 0
    for (rh, rw) in RATIOS:
        hh = SIZE - rh + 1
        ww = SIZE - rw + 1
        for xi in range(hh):
            for yi in range(ww):
                wg = wbase + xi * ww + yi
                for a in range(rh):
                    for b in range(rw):
                        wm[(xi + a) * SIZE + (yi + b), wg] = 1.0 / (rh * rw)
        wbase += hh * ww

    # Suppression table, per group, reproducing the reference IoU math in f32.
    stab = np.zeros((3 * WPAD, WPAD), dtype=np.float32)
    for g in range(3):
        lo, hi = GROUP_BOUNDS[g], GROUP_BOUNDS[g + 1]
        c = coords[lo:hi]
        a = areas[lo:hi]
        wgn = hi - lo
        x1 = np.maximum(c[:, None, 0], c[None, :, 0])
        y1 = np.maximum(c[:, None, 1], c[None, :, 1])
        x2 = np.minimum(c[:, None, 2], c[None, :, 2])
        y2 = np.minimum(c[:, None, 3], c[None, :, 3])
        lw = (x2 - x1 + np.float32(1.0)).astype(np.float32)
        lh = (y2 - y1 + np.float32(1.0)).astype(np.float32)
        inter = np.where((lw < 0) | (lh < 0), np.float32(0.0),
                         (lw * lh).astype(np.float32))
        union = (a[:, None] + a[None, :] - inter).astype(np.float32)
        iou = (inter / union).astype(np.float32)
        sup = (iou > np.float32(IOU_THRESH)) | np.eye(wgn, dtype=bool)
        stab[g * WPAD:g * WPAD + wgn, :wgn] = np.where(sup, np.float32(NEG),
                                                       np.float32(0.0))

    ones = np.ones((128, 1), dtype=np.float32)
    ident = np.eye(8, dtype=np.float32)
    goff = np.zeros((NROWS, 1), dtype=np.uint32)
    for g in range(3):
        goff[32 * g:32 * (g + 1)] = g * WPAD
    return wm, stab, ones, ident, goff


_CONSTS = None
_NC = None


def _get_consts():
    global _CONSTS
    if _CONSTS is None:
        _CONSTS = _build_consts()
    return _CONSTS


def _build_nc():
    from contextlib import ExitStack
    import concourse.bass as bass
    import concourse.bacc as bacc
    from concourse import mybir
    from concourse.tile import TileContext

    f32 = mybir.dt.float32
    u32 = mybir.dt.uint32

    # Bacc (not plain Bass): its finalize() runs move_matmul_waits_to_ldweights
    # + generate_event_semaphores, which split multi-sem waits into the 1-wait-
    # per-instruction form this walrus build requires.
    nc = bacc.Bacc()
    x_p = nc.declare_dram_parameter("x", [NB, C, H, W], f32, isOutput=False)
    wm_p = nc.declare_dram_parameter("wm", [HW, 917], f32, isOutput=False)
    st_p = nc.declare_dram_parameter("stab", [3 * WPAD, WPAD], f32,
                                     isOutput=False)
    on_p = nc.declare_dram_parameter("ones", [128, 1], f32, isOutput=False)
    id_p = nc.declare_dram_parameter("ident", [8, 8], f32, isOutput=False)
    go_p = nc.declare_dram_parameter("goff", [NROWS, 1], u32, isOutput=False)
    osc_p = nc.declare_dram_parameter("out_scores", [NB, 917], f32,
                                      isOutput=True)
    omx_p = nc.declare_dram_parameter("out_maxs", [NROWS, 24], f32,
                                      isOutput=True)
    oix_p = nc.declare_dram_parameter("out_idxs", [NROWS, 24], u32,
                                      isOutput=True)

    with ExitStack() as ctx:
        tc = ctx.enter_context(TileContext(nc))
        consts = ctx.enter_context(tc.tile_pool(name="consts", bufs=1))
        xpool = ctx.enter_context(tc.tile_pool(name="xp", bufs=4))
        small = ctx.enter_context(tc.tile_pool(name="small", bufs=2))
        ps1 = ctx.enter_context(tc.tile_pool(name="ps1", bufs=2, space="PSUM"))
        ps2 = ctx.enter_context(tc.tile_pool(name="ps2", bufs=3, space="PSUM"))
        psT = ctx.enter_context(tc.tile_pool(name="psT", bufs=1, space="PSUM"))
        psW = ctx.enter_context(tc.tile_pool(name="psW", bufs=1, space="PSUM"))

        # Constants into SBUF via SWDGE (gpsimd) so the 8 HWDGE semaphore
        # lanes stay exclusive to the x-batch + output DMAs (lane reuse would
        # add a second sync wait, and most instruction types allow only one).
        wm0 = consts.tile([128, 917], f32, tag="wm0")
        nc.gpsimd.dma_start(out=wm0, in_=wm_p[0:128, :])
        wm1 = consts.tile([68, 917], f32, tag="wm1")
        nc.gpsimd.dma_start(out=wm1, in_=wm_p[128:196, :])
        ones_sb = consts.tile([128, 1], f32, tag="ones")
        nc.gpsimd.dma_start(out=ones_sb, in_=on_p[:])
        ident_sb = consts.tile([8, 8], f32, tag="ident")
        nc.gpsimd.dma_start(out=ident_sb, in_=id_p[:])
        goff_sb = consts.tile([NROWS, 1], u32, tag="goff")
        nc.gpsimd.dma_start(out=goff_sb, in_=go_p[:])

        sc96 = consts.tile([NROWS, WPAD], f32, tag="sc96")
        nc.vector.memset(sc96, NEG)
        # Dummy DVE read of goff so later DVE ops don't need a second wait.
        gjunk = consts.tile([NROWS, 1], u32, tag="gjunk")
        nc.vector.tensor_copy(out=gjunk, in_=goff_sb)
        s_all = consts.tile([1, NB, HW], f32, tag="s_all")
        scores_sb = consts.tile([NB, 917], f32, tag="scores")
        maxs_t = consts.tile([NROWS, 24], f32, tag="maxs")
        idxs_t = consts.tile([NROWS, 24], u32, tag="idxs")

        xv = x_p[:].rearrange("b (p r) h w -> p b r (h w)", p=128)

        # Dummy matmuls so PE observes each const-DMA semaphore before the
        # real matmuls (PE LDWEIGHTS supports only ONE sync wait per
        # instruction, so a matmul can't wait on a const lane AND data).
        pw = psW.tile([8, 8], f32, tag="warm")
        nc.tensor.matmul(out=pw[0:1, 0:1], lhsT=ones_sb, rhs=ones_sb,
                         start=True, stop=True)
        nc.tensor.matmul(out=pw[0:8, 0:1], lhsT=ident_sb,
                         rhs=ident_sb[:, 0:1], start=True, stop=True)
        nc.tensor.matmul(out=pw[0:1, 1:2], lhsT=wm0[:, 0:1],
                         rhs=wm0[:, 0:1], start=True, stop=True)
        nc.tensor.matmul(out=pw[0:1, 2:3], lhsT=wm1[:, 0:1],
                         rhs=wm1[:, 0:1], start=True, stop=True)

        # ---- Stage 1: channel sums -> s_all [8, 196]
        for t in range(4):
            xt = xpool.tile([128, 2, 16, HW], f32, tag="x")
            nc.sync.dma_start(out=xt, in_=xv[:, 2 * t:2 * t + 2])
            for i in range(2):
                b = 2 * t + i
                # DVE folds chunks 0..7 down into chunk 0 (pairwise tree)
                nc.vector.tensor_add(out=xt[:, i, 0:4, :],
                                     in0=xt[:, i, 0:4, :],
                                     in1=xt[:, i, 4:8, :])
                nc.vector.tensor_add(out=xt[:, i, 0:2, :],
                                     in0=xt[:, i, 0:2, :],
                                     in1=xt[:, i, 2:4, :])
                nc.vector.tensor_add(out=xt[:, i, 0:1, :],
                                     in0=xt[:, i, 0:1, :],
                                     in1=xt[:, i, 1:2, :])
                # PE: chunks 8..15 directly + folded chunk 0 -> psum [1, 196]
                pb = ps1.tile([1, HW], f32, tag="pb")
                for j in range(8):
                    nc.tensor.matmul(out=pb, lhsT=ones_sb,
                                     rhs=xt[:, i, 8 + j, :],
                                     start=(j == 0), stop=False)
                nc.tensor.matmul(out=pb, lhsT=ones_sb, rhs=xt[:, i, 0, :],
                                 start=False, stop=True)
                nc.vector.tensor_copy(out=s_all[0:1, b, :], in_=pb)

        # ---- Stage 2: window scores
        pT0 = psT.tile([128, NB], f32, tag="pT0")
        pT1 = psT.tile([68, NB], f32, tag="pT1")
        for b in range(NB):
            nc.tensor.transpose(out=pT0[:, b:b + 1], in_=s_all[0:1, b, 0:128],
                                identity=ident_sb[0:1, 0:1])
            nc.tensor.transpose(out=pT1[:, b:b + 1], in_=s_all[0:1, b, 128:196],
                                identity=ident_sb[0:1, 0:1])
        sT0 = consts.tile([128, NB], f32, tag="sT0")
        nc.vector.tensor_copy(out=sT0, in_=pT0)
        sT1 = consts.tile([68, NB], f32, tag="sT1")
        nc.vector.tensor_copy(out=sT1, in_=pT1)

        for g in range(3):
            lo, hi = GROUP_BOUNDS[g], GROUP_BOUNDS[g + 1]
            wg = hi - lo
            pg = ps2.tile([NB, 512], f32, tag="pg")
            nc.tensor.matmul(out=pg[:, :wg], lhsT=sT0, rhs=wm0[:, lo:hi],
                             start=True, stop=False)
            nc.tensor.matmul(out=pg[:, :wg], lhsT=sT1, rhs=wm1[:, lo:hi],
                             start=False, stop=True)
            nc.vector.tensor_copy(out=scores_sb[:, lo:hi], in_=pg[:, :wg])
            nc.scalar.copy(out=sc96[32 * g:32 * g + NB, 0:wg], in_=pg[:, :wg])
        nc.sync.dma_start(out=osc_p[:], in_=scores_sb)

        # ---- NMS: 3 rounds over sc96
        for k in range(3):
            mx = maxs_t[:, 8 * k:8 * (k + 1)]
            ix = idxs_t[:, 8 * k:8 * (k + 1)]
            nc.vector.max(out=mx, in_=sc96)
            nc.vector.max_index(out=ix, in_max=mx, in_values=sc96)
            if k < 2:
                idxg = small.tile([NROWS, 1], u32, tag="idxg")
                nc.vector.tensor_tensor(out=idxg, in0=ix[:, 0:1],
                                        in1=goff_sb,
                                        op=mybir.AluOpType.add)
                pen = small.tile([NROWS, WPAD], f32, tag="pen")
                nc.gpsimd.indirect_dma_start(
                    out=pen, out_offset=None, in_=st_p[:],
                    in_offset=bass.IndirectOffsetOnAxis(ap=idxg[:, 0:1],
                                                        axis=0))
                nc.vector.tensor_add(out=sc96, in0=sc96, in1=pen)

        nc.sync.dma_start(out=omx_p[:], in_=maxs_t)
        nc.sync.dma_start(out=oix_p[:], in_=idxs_t)

    return nc


def _get_nc():
    global _NC
    if _NC is None:
        _NC = _build_nc()
        if not _NC.is_finalized():
            _NC.finalize()
    return _NC


def kernel(x):
    from concourse.bass_utils import run_bass_kernel_spmd

    x = np.ascontiguousarray(np.asarray(x), dtype=np.float32)
    assert x.shape == (B_TOT, C, H, W)
    wm, stab, ones, ident, goff = _get_consts()
    nc = _get_nc()

    in_maps = []
    for ci in range(NCORES):
        in_maps.append({
            "x": x[ci * NB:(ci + 1) * NB],
            "wm": wm, "stab": stab, "ones": ones,
            "ident": ident, "goff": goff,
        })
    res = run_bass_kernel_spmd(nc, in_maps, core_ids=list(range(NCORES)))
    results = res.results

    wscores = np.concatenate([results[ci]["out_scores"]
                              for ci in range(NCORES)], axis=0)
    pidx = np.zeros((B_TOT, 7), dtype=np.int32)
    pscores = np.zeros((B_TOT, 7), dtype=np.float32)
    col_map = [(g, k) for g in range(3) for k in range(N_LIST[g])]
    for ci in range(NCORES):
        mx = results[ci]["out_maxs"]
        ixs = results[ci]["out_idxs"]
        for b in range(NB):
            for col, (g, k) in enumerate(col_map):
                pidx[ci * NB + b, col] = (int(ixs[32 * g + b, 8 * k])
                                          + GROUP_BOUNDS[g])
                pscores[ci * NB + b, col] = mx[32 * g + b, 8 * k]
    return pidx, pscores, wscores
